# revision 13
# baseline (speedup 1.0000x reference)
"""Trainium2 Bass kernel v2 for nn_AddTaskModel (adaptive-threshold spiking RNN).

Data-parallel over 8 NeuronCores: batch 2048 -> 256/core -> 2 staggered
blocks of 128.  Layout [H=128 partitions, batch on free dim].

v2 changes vs baseline:
  - x-projection on device: psX = W1x[:, :2]@x via one K=6 bf16 matmul
    (hi/lo split of both x and W -> fp32-exact), killing the 1 GB host
    dense_x precompute + DMA and 2 identity matmuls per block-step.
  - state in (bb, m' = mem - B_J0) coordinates: B_J0 folds into the ACT
    copy bias and the sigmoid biases, saving one elementwise op.
  - elementwise restructure: 8 ops/block-step (u,v,pp,m1,T,bb_n,m'_n,spk),
    all on DVE.  (GpSimd offload was tried and reverted: its ~470 ns/op +
    queue latency stretches PE idle gaps past the ~3.4 us HAM window and
    drops the PE to 1.2 GHz -- every GpSimd variant measured slower.)
  - denormal flush amortized to every 8 steps.
  - HAM warm-up preamble: 48 back-to-back LDWEIGHTS before the scan give
    the PE clock-gate the unbroken ~3.4 us activity burst it needs to lift
    to 2.4 GHz; the steady-state loop alone never produces one, and a cold
    PE doubles every matmul (measured 8.2 ms vs 4.36 ms).

Math per step (per batch column, H-vector ops):
  dense' = Wxs@spk + Wxd@x + (b1x - B_J0)            [PE + ACT]
  tauM = sigmoid(Wma@dense' + Wmb@m' + btM')         [PE + ACT]
  tauA = sigmoid(Waa@dense' + Wab@bb + btA')         [PE + ACT]
  u = bb - spk ; v = m' - dense'                     [GpS]
  [p1|w] = [tauA|tauM] * [u|v]                       [DVE]
  m1 = dense' + w ; T = (p1 + (B+J)/B)*spk           [DVE]
  bb_n = p1 + spk                                    [GpS]
  m'_n = m1 - BETA*T                                 [DVE]
  spk_n = (BETA*bb_n) < m'_n                         [DVE]
"""
import sys
if "/opt/trn_rl_repo" not in sys.path:
    sys.path.insert(0, "/opt/trn_rl_repo")

import numpy as np
import ml_dtypes
import concourse.bass as bass
import concourse.mybir as mybir
from concourse import tile

F32 = mybir.dt.float32
BF16 = mybir.dt.bfloat16
ALU = mybir.AluOpType
AF = mybir.ActivationFunctionType

B_J0 = 0.01
BETA = 1.8
C2 = float((BETA + B_J0) / BETA)

H = 128            # hidden
S = 1024           # timesteps
B_FULL = 2048      # full batch
N_CORES = 8
BSH = B_FULL // N_CORES   # 256 per core
FB = 128                  # batch per block
NBLK = 2
X_CHUNK = 16              # steps per xrows DMA chunk
FLUSH_EVERY = 8

NW32 = 4 * H + 3 + 2 * NBLK * 2 * FB  # WmaT|WmbT|WaaT|WabT|btM|btA|dbias|st0*2|sd0*2
NWBF = 2 * H                          # WxsT_hi|WxsT_lo


def _split_multiwaits(nc, max_waits=1):
    """Walrus codegen allows at most one sync wait per instruction; hoist
    extras into standalone EventSemaphore waits on the same engine queue."""
    for f in nc.m.functions:
        for blk in f.blocks:
            newlist = []
            for ins in blk.instructions:
                si = ins.sync_info
                if si is not None and si.on_wait and len(si.on_wait) > max_waits:
                    waits = list(si.on_wait)
                    for k, w in enumerate(waits[:-max_waits]):
                        ev = mybir.InstEventSemaphore(
                            name=f"{ins.name}_xw{k}", ins=[], outs=[])
                        ev.engine = ins.engine
                        ev.sync_info = mybir.SyncInfo(on_wait=[w], on_update=[])
                        newlist.append(ev)
                    ins.sync_info = mybir.SyncInfo(
                        on_wait=waits[-max_waits:],
                        on_update=list(si.on_update or []))
                newlist.append(ins)
            blk.instructions = newlist


def _build_nc(n_steps=S, x_chunk=X_CHUNK, split_multiwaits=True, use_gpsimd=True,
              keepwarm=2, warmup=48, rewarm_period=0, rewarm_count=12):
    nc = bass.Bass()
    xdr = nc.declare_dram_parameter("xrows", [6, n_steps * BSH], BF16, isOutput=False)
    w32 = nc.declare_dram_parameter("wpack32", [H, NW32], F32, isOutput=False)
    wbf = nc.declare_dram_parameter("wpackbf", [H, NWBF], BF16, isOutput=False)
    x4w = nc.declare_dram_parameter("x4w", [6, H], BF16, isOutput=False)
    st_out = nc.declare_dram_parameter("st_out", [H, NBLK * 2 * FB], F32, isOutput=True)

    with tile.TileContext(nc) as tc:
        with (
            tc.tile_pool(name="const", bufs=1) as constp,
            tc.tile_pool(name="xin", bufs=3) as xinp,
            tc.tile_pool(name="state", bufs=3) as statep,
            tc.tile_pool(name="sd", bufs=3) as sdp,
            tc.tile_pool(name="tau", bufs=3) as taup,
            tc.tile_pool(name="uv", bufs=3) as uvp,
            tc.tile_pool(name="pp", bufs=3) as ppp,
            tc.tile_pool(name="tmp", bufs=3) as tmpp,
            tc.tile_pool(name="ps", bufs=1, space="PSUM") as psp,
            tc.tile_pool(name="psx", bufs=2, space="PSUM") as psxp,
        ):
            wsb = constp.tile([H, NW32], F32)
            nc.sync.dma_start(wsb[:], w32[:])
            w_ma = wsb[:, 0 * H:1 * H]
            w_mb = wsb[:, 1 * H:2 * H]
            w_aa = wsb[:, 2 * H:3 * H]
            w_ab = wsb[:, 3 * H:4 * H]
            btM = wsb[:, 4 * H + 0:4 * H + 1]
            btA = wsb[:, 4 * H + 1:4 * H + 2]
            dbias = wsb[:, 4 * H + 2:4 * H + 3]
            c0 = 4 * H + 3
            st = [wsb[:, c0 + b * 2 * FB: c0 + (b + 1) * 2 * FB] for b in range(NBLK)]
            c1 = c0 + NBLK * 2 * FB
            # sd = [spk | dense'] per block; spk stored fp32 ({0,1} exact);
            # the matmuls read its top-16-bits as an exact bf16 view
            sd = [wsb[:, c1 + b * 2 * FB: c1 + (b + 1) * 2 * FB] for b in range(NBLK)]

            bsb = constp.tile([H, NWBF], BF16)
            nc.sync.dma_start(bsb[:], wbf[:])
            w_xh = bsb[:, 0 * H:1 * H]
            w_xl = bsb[:, 1 * H:2 * H]

            xw = constp.tile([6, H], BF16)
            nc.sync.dma_start(xw[:], x4w[:])

            # HAM warm-up: the PE clock-gate only lifts to 2.4 GHz after a
            # ~3.4 us UNBROKEN burst of PE activity; the steady-state loop
            # never produces one.  Burn one burst of back-to-back weight
            # loads up front so the whole scan runs at full clock.
            for _ in range(warmup):
                nc.tensor.ldweights(w_xh)

            gps = nc.gpsimd if use_gpsimd else nc.vector
            xcnk = None
            for t in range(n_steps):
                if t % x_chunk == 0:
                    n_st = min(x_chunk, n_steps - t)
                    xcnk = xinp.tile([6, x_chunk * BSH], BF16, tag="x", name=f"x_{t}")
                    nc.sync.dma_start(xcnk[:, 0:n_st * BSH],
                                      xdr[:, t * BSH:(t + n_st) * BSH])
                if rewarm_period and t % rewarm_period == rewarm_period - 1:
                    # periodic unbroken LDW burst: re-warms the HAM clock-gate
                    # if a long stall dropped the PE to 1.2 GHz mid-run
                    for _ in range(rewarm_count):
                        nc.tensor.ldweights(w_xh)
                psmv = [None] * NBLK
                psav = [None] * NBLK
                sdnv = [None] * NBLK
                # phase 1 (both blocks): psX mms, dense' copy, tau-preact mms.
                for b in range(NBLK):
                    bb = st[b][:, 0:FB]
                    mp = st[b][:, FB:2 * FB]
                    # exact bf16 view of the fp32 spike half (values {0,1})
                    spk_bf = sd[b][:, 0:FB].bitcast(BF16)[:, 1::2]
                    co = (t % x_chunk) * BSH + b * FB
                    xt = xcnk[:, co:co + FB]

                    # psX = Wxd@x (K=6 exact) + Wxs_hi@spk + Wxs_lo@spk
                    psx = psxp.tile([H, FB], F32, tag=f"psX{b}", name=f"psX{b}_{t}")
                    nc.tensor.matmul(psx[:], xw[:], xt, start=True, stop=False)
                    if keepwarm:
                        for _ in range(keepwarm):
                            nc.tensor.ldweights(w_xh[0:32, :])
                    nc.tensor.matmul(psx[:], w_xh, spk_bf, start=False, stop=False)
                    nc.tensor.matmul(psx[:], w_xl, spk_bf, start=False, stop=True)

                    # dense' = psX + (b1x - B_J0), into sd right half
                    nc.scalar.activation(sd[b][:, FB:2 * FB], psx[:], AF.Identity,
                                         bias=dbias)

                    psm = psp.tile([H, FB], F32, tag=f"psM{b}", name=f"psM{b}_{t}")
                    psa = psp.tile([H, FB], F32, tag=f"psA{b}", name=f"psA{b}_{t}")
                    nc.tensor.matmul(psm[:], w_mb, mp, start=True, stop=False)
                    nc.tensor.matmul(psa[:], w_ab, bb, start=True, stop=False)
                    if keepwarm:
                        for _ in range(keepwarm):
                            nc.tensor.ldweights(w_xl[0:32, :])
                    nc.tensor.matmul(psm[:], w_ma, sd[b][:, FB:2 * FB],
                                     start=False, stop=True)
                    nc.tensor.matmul(psa[:], w_aa, sd[b][:, FB:2 * FB],
                                     start=False, stop=True)
                    psmv[b] = psm
                    psav[b] = psa

                # phase 2 (both blocks): sigmoids + elementwise update
                for b in range(NBLK):
                    spk_f = sd[b][:, 0:FB]

                    tauMA = taup.tile([H, 2 * FB], F32, tag=f"tau{b}")  # [tauA|tauM]
                    nc.scalar.activation(tauMA[:, FB:2 * FB], psmv[b][:], AF.Sigmoid,
                                         bias=btM)
                    nc.scalar.activation(tauMA[:, 0:FB], psav[b][:], AF.Sigmoid,
                                         bias=btA)

                    # uv = [bb|m'] - [spk|dense'] = [u|v]
                    uv = uvp.tile([H, 2 * FB], F32, tag=f"uv{b}")
                    nc.vector.tensor_tensor(uv[:], st[b][:], sd[b][:], ALU.subtract)
                    # pp = [tauA*u | tauM*v] = [p1|w]
                    pp = ppp.tile([H, 2 * FB], F32, tag=f"pp{b}")
                    nc.vector.tensor_tensor(pp[:], tauMA[:], uv[:], ALU.mult)
                    p1 = pp[:, 0:FB]

                    Tt = tmpp.tile([H, FB], F32, tag=f"T{b}")
                    nc.vector.scalar_tensor_tensor(Tt[:], p1, C2, spk_f,
                                                   ALU.add, ALU.mult)
                    # bm = pp + sd = [p1+spk | w+dense'] = [bb_n | m1]
                    bm = statep.tile([H, 2 * FB], F32, tag=f"st{b}")
                    nc.vector.tensor_tensor(bm[:], pp[:], sd[b][:], ALU.add)
                    # m'_n = m1 - BETA*T, in place over m1
                    nc.vector.scalar_tensor_tensor(bm[:, FB:2 * FB], Tt[:], -BETA,
                                                   bm[:, FB:2 * FB], ALU.mult, ALU.add)
                    # spike into the NEXT sd tile's left half, as fp32 {0,1}
                    sd_n = sdp.tile([H, 2 * FB], F32, tag=f"sd{b}")
                    nc.vector.scalar_tensor_tensor(sd_n[:, 0:FB], bm[:, 0:FB], BETA,
                                                   bm[:, FB:2 * FB], ALU.mult, ALU.is_lt)
                    if t % FLUSH_EVERY == FLUSH_EVERY - 1:
                        nc.vector.tensor_scalar(bm[:, 0:FB], bm[:, 0:FB], 1e-10,
                                                None, ALU.max)

                    st[b] = bm
                    sdnv[b] = sd_n
                for b in range(NBLK):
                    sd[b] = sdnv[b]

            for b in range(NBLK):
                nc.sync.dma_start(st_out[:, b * 2 * FB:(b + 1) * 2 * FB], st[b][:])

    if split_multiwaits:
        _split_multiwaits(nc)
    return nc


def _bf16_split(a):
    hi = a.astype(ml_dtypes.bfloat16)
    lo = (a - hi.astype(np.float32)).astype(ml_dtypes.bfloat16)
    return hi, lo


def _prep_inputs_per_core(inputs, n_steps=S):
    x = np.asarray(inputs["x"], np.float32)          # [S, B, 2]
    W1x = np.asarray(inputs["W1x"], np.float32)
    b1x = np.asarray(inputs["b1x"], np.float32)
    WtauM = np.asarray(inputs["WtauM"], np.float32)
    WtauAdp = np.asarray(inputs["WtauAdp"], np.float32)
    btauM = np.asarray(inputs["btauM"], np.float32)
    btauAdp = np.asarray(inputs["btauAdp"], np.float32)
    h0_mem = np.asarray(inputs["h0_mem"], np.float32)
    h0_spk = np.asarray(inputs["h0_spk"], np.float32)
    h0_b = np.asarray(inputs["h0_b"], np.float32)

    # folded biases
    btM = btauM + B_J0 * (WtauM[:, :H].sum(1) + WtauM[:, H:].sum(1))
    btA = btauAdp + B_J0 * WtauAdp[:, :H].sum(1)
    dbias = b1x - B_J0

    # x4w stationary [6, H]: rows pair with xrows moving rows
    Wx0, Wx1 = W1x[:, 0], W1x[:, 1]
    W0h, W0l = _bf16_split(Wx0)
    W1h, W1l = _bf16_split(Wx1)
    x4w = np.stack([np.asarray(W0h), np.asarray(W1h), np.asarray(W0h),
                    np.asarray(W1h), np.asarray(W0l), np.asarray(W1l)],
                   axis=0).astype(ml_dtypes.bfloat16)   # [6, H]

    wxsT = np.ascontiguousarray(W1x[:, 2:].T, np.float32)
    wxs_hi, wxs_lo = _bf16_split(wxsT)

    wcols = [WtauM[:, :H].T, WtauM[:, H:].T, WtauAdp[:, :H].T, WtauAdp[:, H:].T,
             btM[:, None], btA[:, None], dbias[:, None]]

    in_maps = []
    for c in range(N_CORES):
        sl = slice(c * BSH, (c + 1) * BSH)
        xs = x[:n_steps, sl, :]                       # [S, 256, 2]
        x0h, x0l = _bf16_split(xs[:, :, 0])
        x1h, x1l = _bf16_split(xs[:, :, 1])
        xrows = np.stack([x0h, x1h, x0l, x1l, x0h, x1h], axis=0)  # [6, S, 256]
        m = {
            "xrows": np.ascontiguousarray(
                xrows.reshape(6, n_steps * BSH)).astype(ml_dtypes.bfloat16),
            "x4w": x4w,
        }
        p32 = list(wcols)
        for b in range(NBLK):
            bsl = slice(c * BSH + b * FB, c * BSH + (b + 1) * FB)
            p32.append(np.concatenate(
                [h0_b[bsl].T, (h0_mem[bsl] - B_J0).T], axis=1))   # st0 [H, 256]
        for b in range(NBLK):
            bsl = slice(c * BSH + b * FB, c * BSH + (b + 1) * FB)
            p32.append(np.concatenate(
                [h0_spk[bsl].T, np.zeros((H, FB), np.float32)], axis=1))  # sd0
        m["wpack32"] = np.ascontiguousarray(
            np.concatenate(p32, axis=1).astype(np.float32))
        m["wpackbf"] = np.ascontiguousarray(
            np.concatenate([np.asarray(a, dtype=ml_dtypes.bfloat16)
                            for a in (wxs_hi, wxs_lo)], axis=1))
        in_maps.append(m)
    return in_maps


_NC_CACHE = {}


BUILD_KWARGS = dict(use_gpsimd=False, keepwarm=0, warmup=48,
                    rewarm_period=8, rewarm_count=12)


def _get_nc():
    if "nc" not in _NC_CACHE:
        _NC_CACHE["nc"] = _build_nc(**BUILD_KWARGS)
    return _NC_CACHE["nc"]


def _run(inputs, trace=False):
    from concourse.bass_utils import run_bass_kernel_spmd
    nc = _get_nc()
    in_maps = _prep_inputs_per_core(inputs)
    res = run_bass_kernel_spmd(nc, in_maps, core_ids=list(range(N_CORES)),
                               trace=trace)
    return res


def _finish_host(results, inputs):
    Wlin = np.asarray(inputs["Wlin"], np.float32)
    blin = np.asarray(inputs["blin"], np.float32)
    y = np.asarray(inputs["y"], np.float32)
    mems = []
    for r in results:
        so = r["st_out"]                              # [H, 512]
        for b in range(NBLK):
            mems.append(so[:, b * 2 * FB + FB:(b + 1) * 2 * FB].T + B_J0)
    mem = np.concatenate(mems, axis=0)                # [B, H]
    out = (mem @ Wlin.T + blin)[:, 0]
    return np.float32(np.mean((out.astype(np.float32) - y) ** 2, dtype=np.float32))


def kernel(x, y, h0_mem, h0_spk, h0_b, W1x, b1x, WtauM, btauM, WtauAdp,
           btauAdp, Wlin, blin):
    """Full (unsharded) inputs -> full scalar loss, computed on 8 TRN2 cores."""
    inputs = dict(x=x, y=y, h0_mem=h0_mem, h0_spk=h0_spk, h0_b=h0_b,
                  W1x=W1x, b1x=b1x, WtauM=WtauM, btauM=btauM,
                  WtauAdp=WtauAdp, btauAdp=btauAdp, Wlin=Wlin, blin=blin)
    res = _run(inputs, trace=False)
    return _finish_host(res.results, inputs)


def kernel_profiled(**inputs):
    """Like kernel(), but also returns neuron-profile exec time in ns."""
    res = _run(inputs, trace=True)
    return _finish_host(res.results, inputs), res.exec_time_ns
